# revision 16
# baseline (speedup 1.0000x reference)
"""Trainium2 Bass kernel for a transformer decoder layer (self-attn + cross-attn + FFN).

Contract: kernel(**inputs) takes the FULL unsharded inputs (as produced by
setup_inputs()) and returns the FULL outputs (out3, aw1, aw2), matching the
reference. Internally the work is sharded across 8 NeuronCores:
  core c -> batch b = c//4, row-group qi = c%4 owning the four strided
  128-row q-tiles {qi, qi+4, qi+8, qi+12} of the sequence.
Every core runs the SAME program; all per-core differences live in the data
slices passed via in_maps. No collectives are needed: K/V projections are
computed per-batch on each core (cheap), while attention rows, layernorms,
FFN rows and all outputs are disjoint per core.

Precision: matmul operands in bf16 (f32 PSUM accumulate), softmax in f32
without max subtraction (valid logits are bounded ~|8|), attention-weight
outputs written as f32 = bf16(exp) * f32(1/sum).
"""

import numpy as np

import concourse.bass as bass
import concourse.tile as tile
import concourse.mybir as mybir
from concourse import bacc
from concourse.bass_utils import run_bass_kernel_spmd
from concourse.masks import make_identity

F32 = mybir.dt.float32
BF16 = mybir.dt.bfloat16
AF = mybir.ActivationFunctionType
OP = mybir.AluOpType

B, S, DM, H, HID = 2, 2048, 512, 8, 2048
D = DM // H          # 64
NEG = -1e9
EPS = 1e-6
NC_COUNT = 8
NT = 4               # q-tiles per core (128 rows each)
P = 128
KC = DM // P         # 4 contraction chunks of 128 over DM
HC = HID // P        # 16 chunks over HID
ST = S // P          # 16 seq tiles


def _build():
    nc = bacc.Bacc("TRN2", target_bir_lowering=False, debug=False,
                   num_devices=NC_COUNT)

    dram = {}

    def din(name, shape):
        dram[name] = nc.dram_tensor(name, shape, F32, kind="ExternalInput").ap()

    def dout(name, shape):
        dram[name] = nc.dram_tensor(name, shape, F32, kind="ExternalOutput").ap()

    din("xb", [S, DM])          # x[b] (for K1/V1 projections)
    din("xq", [NT * P, DM])     # x[b, own rows] (Q1 + residual 1)
    din("encb", [S, DM])        # enc_output[b]
    din("bmask", [NT, P, 512])  # look-ahead mask boundary chunk per tile (raw 0/1)
    din("pm", [1, S])           # padding_mask row
    for l in ("1", "2"):
        for w in ("wq", "wk", "wv", "wo"):
            din(f"mha{l}_{w}", [DM, DM])
        for b_ in ("bq", "bk", "bv", "bo"):
            din(f"mha{l}_{b_}", [DM])
    din("ffn_w1", [DM, HID])
    din("ffn_b1", [HID])
    din("ffn_w2", [HID, DM])
    din("ffn_b2", [DM])
    for j in ("1", "2", "3"):
        din(f"ln{j}_g", [DM])
        din(f"ln{j}_b", [DM])

    dout("aw1_s", [H, NT, P, S])
    dout("aw2_s", [H, NT, P, S])
    dout("out3_s", [NT, P, DM])

    with tile.TileContext(nc) as tc:
        _emit(tc, dram)
    nc.compile()
    return nc


def _emit(tc, dram):
    nc = tc.nc
    vec = nc.vector
    act = nc.scalar
    gp = nc.gpsimd

    def load_w_bf16(pool, name, kchunks, ncols):
        # DRAM [kchunks*128, ncols] f32 -> SBUF [128, kchunks, ncols] bf16
        t = pool.tile([P, kchunks, ncols], BF16, tag=f"w_{name}")
        gp.dma_start(t[:], dram[name].rearrange("(c p) n -> p c n", p=P))
        return t

    def load_bias_cols(pool, name, nch, scale=None):
        # DRAM [nch*128] f32 -> SBUF [128, nch] f32 (column j = chunk j)
        t = pool.tile([P, nch], F32, tag=f"b_{name}")
        nc.sync.dma_start(t[:], dram[name].rearrange("(c p) -> p c", p=P))
        if scale is not None:
            act.mul(t[:], t[:], scale)
        return t

    def bcast_row(pool, name, width=DM):
        # DRAM [width] f32 -> [128, width] f32 replicated across partitions
        row = pool.tile([1, width], F32, tag=f"r_{name}")
        nc.sync.dma_start(row[:], dram[name][:])
        t = pool.tile([P, width], F32, tag=f"bc_{name}")
        gp.partition_broadcast(t[:], row[:])
        return t

    def stage_transposed(pool, psname, src_ap, nseq, tag, ident_bf):
        # [nseq, DM] f32 DRAM -> bf16 [128 dm-part, KC, nseq] (i.e. x^T)
        dst = pool.tile([P, KC, nseq], BF16, tag=f"T_{tag}")
        nst = nseq // P
        src3 = src_ap.rearrange("(t p) n -> p t n", p=P)
        with tc.tile_pool(name=f"stg_{psname}", bufs=2) as sp, \
             tc.tile_pool(name=f"tp_{psname}", bufs=2, space="PSUM") as pp:
            for t in range(nst):
                s_bf = sp.tile([P, DM], BF16, tag="cast")
                gp.dma_start(s_bf[:], src3[:, t, :])
                ps = pp.tile([P, DM], BF16, tag="ps")
                for c in range(KC):
                    nc.tensor.transpose(ps[:, c * P:(c + 1) * P],
                                        s_bf[:, c * P:(c + 1) * P], ident_bf[:])
                for c in range(KC):
                    vec.tensor_copy(dst[:, c, t * P:(t + 1) * P],
                                    ps[:, c * P:(c + 1) * P])
        return dst

    def layernorm(lnp, z, g_bc, c_bc, out_ap):
        s = lnp.tile([P, 1], F32, tag="s")
        vec.reduce_sum(s[:], z[:], axis=mybir.AxisListType.X)
        mean = lnp.tile([P, 1], F32, tag="mean")
        act.mul(mean[:], s[:], 1.0 / DM)
        sq = lnp.tile([P, DM], F32, tag="lnscratch")
        ss = lnp.tile([P, 1], F32, tag="ss")
        act.activation(sq[:], z[:], AF.Square, accum_out=ss[:])
        m2 = lnp.tile([P, 1], F32, tag="m2")
        vec.tensor_mul(m2[:], mean[:], mean[:])
        var = lnp.tile([P, 1], F32, tag="var")
        act.mul(var[:], ss[:], 1.0 / DM)
        vec.tensor_sub(var[:], var[:], m2[:])
        vec.tensor_scalar_add(var[:], var[:], EPS)
        sd = lnp.tile([P, 1], F32, tag="sd")
        act.sqrt(sd[:], var[:])
        rstd = lnp.tile([P, 1], F32, tag="rstd")
        vec.reciprocal(rstd[:], sd[:])
        t1 = lnp.tile([P, DM], F32, tag="lnscratch")
        vec.tensor_scalar(t1[:], z[:], mean[:], rstd[:],
                          op0=OP.subtract, op1=OP.mult)
        vec.tensor_mul(t1[:], t1[:], g_bc[:])
        vec.tensor_add(out_ap, t1[:], c_bc[:])

    # ================= constants (persistent) =================
    with tc.tile_pool(name="consts", bufs=1) as cpool, \
         tc.tile_pool(name="carry", bufs=1) as opool:

        ident_bf = cpool.tile([P, P], BF16, tag="ident_bf")
        make_identity(nc, ident_bf[:])
        ident_f = cpool.tile([P, P], F32, tag="ident_f")
        make_identity(nc, ident_f[:])

        bv1_bc = bcast_row(cpool, "mha1_bv")
        bo1_bc = bcast_row(cpool, "mha1_bo")
        bv2_bc = bcast_row(cpool, "mha2_bv")
        bo2_bc = bcast_row(cpool, "mha2_bo")
        b2f_bc = bcast_row(cpool, "ffn_b2")
        g1_bc = bcast_row(cpool, "ln1_g")
        c1_bc = bcast_row(cpool, "ln1_b")
        g2_bc = bcast_row(cpool, "ln2_g")
        c2_bc = bcast_row(cpool, "ln2_b")
        g3_bc = bcast_row(cpool, "ln3_g")
        c3_bc = bcast_row(cpool, "ln3_b")

        # ---------- shared attention-layer emitter ----------
        def attention_layer(lname, kvT, qT_src, res_of_t, bo_bc, bv_bc,
                            g_bc, cst_bc, aw_out, causal, out_sb,
                            pm_bf=None, bm=None):
            with tc.tile_pool(name=f"wk_{lname}", bufs=1) as wp:
                wq = load_w_bf16(wp, f"mha{lname}_wq", KC, DM)
                wk = load_w_bf16(wp, f"mha{lname}_wk", KC, DM)
                wv = load_w_bf16(wp, f"mha{lname}_wv", KC, DM)
                wo = load_w_bf16(wp, f"mha{lname}_wo", KC, DM)
                bq_cols = load_bias_cols(wp, f"mha{lname}_bq", KC, 0.125)
                bk_cols = load_bias_cols(wp, f"mha{lname}_bk", KC)

                kt_tiles, v_tiles, qt_tiles = [], [], []
                # ---------------- projections ----------------
                with tc.tile_pool(name=f"pjk_{lname}", bufs=1,
                                  space="PSUM") as pjk, \
                     tc.tile_pool(name=f"pjv_{lname}", bufs=2,
                                  space="PSUM") as pjv, \
                     tc.tile_pool(name=f"pjq_{lname}", bufs=1,
                                  space="PSUM") as pjq, \
                     tc.tile_pool(name=f"pjs_{lname}", bufs=2) as pjs:
                    for pr in range(4):   # head pairs
                        # K^T for head pair: [128, S]
                        kps = pjk.tile([P, S], F32, tag="kps")
                        for c in range(KC):
                            for j in range(S // 512):
                                nc.tensor.matmul(
                                    kps[:, j * 512:(j + 1) * 512],
                                    wk[:, c, pr * P:(pr + 1) * P],
                                    kvT[:, c, j * 512:(j + 1) * 512],
                                    start=(c == 0), stop=(c == KC - 1))
                        if causal:
                            kt = wp.tile([P, S], BF16, tag=f"kt_{pr}")
                            vec.tensor_scalar_add(kt[:], kps[:],
                                                  bk_cols[:, pr:pr + 1])
                            kt_tiles.append(kt)
                        else:
                            # pair-aligned PSUM->SBUF, then DMA-shift halves
                            # into per-head [65, S] tiles (row 64 = pad mask)
                            kpair = pjs.tile([P, S], BF16, tag="kpair")
                            vec.tensor_scalar_add(kpair[:], kps[:],
                                                  bk_cols[:, pr:pr + 1])
                            for hf in range(2):
                                kt = wp.tile([65, S], BF16,
                                             tag=f"kt_{pr * 2 + hf}")
                                nc.sync.dma_start(
                                    kt[0:64, :],
                                    kpair[hf * 64:(hf + 1) * 64, :])
                                nc.sync.dma_start(kt[64:65, :], pm_bf[:])
                                kt_tiles.append(kt)
                        # V for head pair: [seq-part, ST, 128]
                        vt = wp.tile([P, ST, P], BF16, tag=f"v_{pr}")
                        v_tiles.append(vt)
                        for st4 in range(ST // 4):
                            vps = pjv.tile([P, 512], F32, tag="vps")
                            for q4 in range(4):
                                st = st4 * 4 + q4
                                for c in range(KC):
                                    nc.tensor.matmul(
                                        vps[:, q4 * P:(q4 + 1) * P],
                                        kvT[:, c, st * P:(st + 1) * P],
                                        wv[:, c, pr * P:(pr + 1) * P],
                                        start=(c == 0), stop=(c == KC - 1))
                            for q4 in range(4):
                                st = st4 * 4 + q4
                                vec.tensor_add(vt[:, st, :],
                                               vps[:, q4 * P:(q4 + 1) * P],
                                               bv_bc[:, pr * P:(pr + 1) * P])
                        # Q^T (own rows), pre-scaled by 1/8
                        qps = pjq.tile([P, NT * P], F32, tag="qps")
                        for c in range(KC):
                            nc.tensor.matmul(qps[:],
                                             wq[:, c, pr * P:(pr + 1) * P],
                                             qT_src[:, c, :],
                                             start=(c == 0), stop=(c == KC - 1))
                        if causal:
                            qt = wp.tile([P, NT * P], BF16, tag=f"qt_{pr}")
                            vec.tensor_scalar(qt[:], qps[:], 0.125,
                                              bq_cols[:, pr:pr + 1],
                                              op0=OP.mult, op1=OP.add)
                            qt_tiles.append(qt)
                        else:
                            qpair = pjs.tile([P, NT * P], BF16, tag="qpair")
                            vec.tensor_scalar(qpair[:], qps[:], 0.125,
                                              bq_cols[:, pr:pr + 1],
                                              op0=OP.mult, op1=OP.add)
                            for hf in range(2):
                                qt = wp.tile([65, NT * P], BF16,
                                             tag=f"qt_{pr * 2 + hf}")
                                nc.sync.dma_start(
                                    qt[0:64, :],
                                    qpair[hf * 64:(hf + 1) * 64, :])
                                vec.memset(qt[64:65, :], 1.0)
                                qt_tiles.append(qt)

                # ---------------- attention + Wo + LN ----------------
                with tc.tile_pool(name=f"at_{lname}", bufs=2,
                                  space="PSUM") as pL, \
                     tc.tile_pool(name=f"pt_{lname}", bufs=1,
                                  space="PSUM") as pT, \
                     tc.tile_pool(name=f"cx_{lname}", bufs=2,
                                  space="PSUM") as pC, \
                     tc.tile_pool(name=f"wo_{lname}", bufs=1,
                                  space="PSUM") as pO, \
                     tc.tile_pool(name=f"sm_{lname}", bufs=2) as sm, \
                     tc.tile_pool(name=f"ln_{lname}", bufs=2) as lnp:
                    for t in range(NT):
                        nv = 512 * (t + 1) if causal else S
                        nb = nv // P
                        ctxT = sm.tile([P, KC, P], BF16, tag="ctxT")
                        for h in range(H):
                            pr, hf = h // 2, h % 2
                            if causal:
                                lhsq = qt_tiles[pr][hf * 64:(hf + 1) * 64,
                                                    t * P:(t + 1) * P]
                            else:
                                lhsq = qt_tiles[h][:, t * P:(t + 1) * P]
                            E = sm.tile([P, S], BF16, tag="E")
                            ssum = sm.tile([P, 1], F32, tag="ssum")
                            # logits + exp in chunks of 1024 (2 PSUM banks)
                            nch = (nv + 1023) // 1024
                            for ch in range(nch):
                                clen = min(1024, nv - ch * 1024)
                                L = pL.tile([P, 1024], F32, tag="L")
                                for j in range(clen // 512):
                                    off = ch * 1024 + j * 512
                                    if causal:
                                        rhsk = kt_tiles[pr][
                                            hf * 64:(hf + 1) * 64,
                                            off:off + 512]
                                    else:
                                        rhsk = kt_tiles[h][:, off:off + 512]
                                    nc.tensor.matmul(
                                        L[:, j * 512:(j + 1) * 512],
                                        lhsq, rhsk, start=True, stop=True)
                                if causal and t * 512 // 1024 == ch:
                                    boff = t * 512 % 1024
                                    vec.tensor_add(L[:, boff:boff + 512],
                                                   L[:, boff:boff + 512],
                                                   bm[:, t, :])
                                sc = sm.tile([P, 1], F32, tag="sc")
                                act.activation(E[:, ch * 1024:ch * 1024 + clen],
                                               L[:, :clen], AF.Exp,
                                               accum_out=sc[:])
                                if ch == 0:
                                    vec.tensor_copy(ssum[:], sc[:])
                                else:
                                    vec.tensor_add(ssum[:], ssum[:], sc[:])
                            rs = sm.tile([P, 1], F32, tag="rs")
                            vec.reciprocal(rs[:], ssum[:])
                            for ch in range(nch):
                                clen = min(1024, nv - ch * 1024)
                                awf = sm.tile([P, 1024], F32, tag="awf")
                                vec.tensor_scalar_mul(
                                    awf[:, :clen],
                                    E[:, ch * 1024:ch * 1024 + clen], rs[:])
                                nc.sync.dma_start(
                                    aw_out[h, t, :,
                                           ch * 1024:ch * 1024 + clen],
                                    awf[:, :clen])
                            pn = sm.tile([P, S], BF16, tag="pn")
                            vec.tensor_scalar_mul(pn[:, :nv], E[:, :nv], rs[:])
                            # ---- P^T via PE, then P @ V ----
                            if hf == 0:
                                cpair = pC.tile([P, P], F32, tag="cps")
                            cps = cpair[hf * 64:(hf + 1) * 64, :]
                            for k4 in range((nb + 3) // 4):
                                tp = pT.tile([P, 512], BF16, tag="tp")
                                kn = min(4, nb - k4 * 4)
                                for q4 in range(kn):
                                    kc = k4 * 4 + q4
                                    nc.tensor.transpose(
                                        tp[:, q4 * P:(q4 + 1) * P],
                                        pn[:, kc * P:(kc + 1) * P],
                                        ident_bf[:])
                                pts = sm.tile([P, 512], BF16, tag="pts")
                                vec.tensor_copy(pts[:, :kn * P],
                                                tp[:, :kn * P])
                                for q4 in range(kn):
                                    kc = k4 * 4 + q4
                                    nc.tensor.matmul(
                                        cps,
                                        v_tiles[pr][:, kc,
                                                    hf * 64:(hf + 1) * 64],
                                        pts[:, q4 * P:(q4 + 1) * P],
                                        start=(kc == 0), stop=(kc == nb - 1))
                            if hf == 1:
                                act.copy(ctxT[:, pr, :], cpair[:])
                        # ---- Wo + residual + LN ----
                        ops = pO.tile([P, DM], F32, tag="ops")
                        for c in range(KC):
                            nc.tensor.matmul(ops[:], ctxT[:, c, :],
                                             wo[:, c, :],
                                             start=(c == 0), stop=(c == KC - 1))
                        z = lnp.tile([P, DM], F32, tag="z")
                        vec.tensor_add(z[:], ops[:], res_of_t(t))
                        vec.tensor_add(z[:], z[:], bo_bc[:])
                        layernorm(lnp, z, g_bc, cst_bc, out_sb[:, t, :])

        # ================= MHA1 (causal self-attention) =================
        out1 = opool.tile([P, NT, DM], F32, tag="out1")
        with tc.tile_pool(name="lay1", bufs=1) as l1p:
            xq_res = l1p.tile([P, NT, DM], F32, tag="xq_res")
            nc.sync.dma_start(xq_res[:],
                              dram["xq"].rearrange("(t p) n -> p t n", p=P))
            bm = l1p.tile([P, NT, 512], F32, tag="bmask")
            nc.sync.dma_start(bm[:], dram["bmask"].rearrange("t p n -> p t n"))
            vec.tensor_scalar_mul(bm[:], bm[:], NEG)
            xT = stage_transposed(l1p, "x", dram["xb"], S, "x", ident_bf)
            xqT = stage_transposed(l1p, "xq", dram["xq"], NT * P, "xq",
                                   ident_bf)
            attention_layer("1", xT, xqT, lambda t: xq_res[:, t, :],
                            bo1_bc, bv1_bc, g1_bc, c1_bc, dram["aw1_s"],
                            True, out1, bm=bm)

        # transpose out1 -> [DM-part, chunk, q] bf16 for Q2 projection
        out1T = opool.tile([P, KC, NT * P], BF16, tag="out1T")
        with tc.tile_pool(name="t1p", bufs=2, space="PSUM") as pp:
            for t in range(NT):
                ps = pp.tile([P, DM], F32, tag="t1ps")
                for c in range(KC):
                    nc.tensor.transpose(ps[:, c * P:(c + 1) * P],
                                        out1[:, t, c * P:(c + 1) * P],
                                        ident_f[:])
                for c in range(KC):
                    vec.tensor_copy(out1T[:, c, t * P:(t + 1) * P],
                                    ps[:, c * P:(c + 1) * P])

        # ================= MHA2 (cross-attention) =================
        out2 = opool.tile([P, NT, DM], F32, tag="out2")
        with tc.tile_pool(name="lay2", bufs=1) as l2p:
            pm_f = l2p.tile([1, S], F32, tag="pm_f")
            nc.sync.dma_start(pm_f[:], dram["pm"][:])
            pm_bf = l2p.tile([1, S], BF16, tag="pm_bf")
            vec.tensor_scalar_mul(pm_bf[:], pm_f[:], NEG)
            encT = stage_transposed(l2p, "enc", dram["encb"], S, "enc",
                                    ident_bf)
            attention_layer("2", encT, out1T, lambda t: out1[:, t, :],
                            bo2_bc, bv2_bc, g2_bc, c2_bc, dram["aw2_s"],
                            False, out2, pm_bf=pm_bf)

        # transpose out2 for FFN
        out2T = opool.tile([P, KC, NT * P], BF16, tag="out2T")
        with tc.tile_pool(name="t2p", bufs=2, space="PSUM") as pp:
            for t in range(NT):
                ps = pp.tile([P, DM], F32, tag="t2ps")
                for c in range(KC):
                    nc.tensor.transpose(ps[:, c * P:(c + 1) * P],
                                        out2[:, t, c * P:(c + 1) * P],
                                        ident_f[:])
                for c in range(KC):
                    vec.tensor_copy(out2T[:, c, t * P:(t + 1) * P],
                                    ps[:, c * P:(c + 1) * P])

        # ================= FFN + LN3 =================
        with tc.tile_pool(name="ffn", bufs=1) as fp:
            w1_sb = load_w_bf16(fp, "ffn_w1", KC, HID)
            w2_sb = load_w_bf16(fp, "ffn_w2", HC, DM)
            b1f = load_bias_cols(fp, "ffn_b1", HC)
            hT = fp.tile([P, HC, NT * P], BF16, tag="hT")
            with tc.tile_pool(name="fh", bufs=2, space="PSUM") as ph:
                for hc in range(HC):
                    hp = ph.tile([P, NT * P], F32, tag="hp")
                    for c in range(KC):
                        nc.tensor.matmul(hp[:],
                                         w1_sb[:, c, hc * P:(hc + 1) * P],
                                         out2T[:, c, :],
                                         start=(c == 0), stop=(c == KC - 1))
                    act.activation(hT[:, hc, :], hp[:], AF.Relu,
                                   bias=b1f[:, hc:hc + 1])
            with tc.tile_pool(name="fo", bufs=2, space="PSUM") as po, \
                 tc.tile_pool(name="ln3", bufs=2) as lnp:
                for t in range(NT):
                    op3 = po.tile([P, DM], F32, tag="op3")
                    for hc in range(HC):
                        nc.tensor.matmul(op3[:],
                                         hT[:, hc, t * P:(t + 1) * P],
                                         w2_sb[:, hc, :],
                                         start=(hc == 0), stop=(hc == HC - 1))
                    z = lnp.tile([P, DM], F32, tag="z3")
                    vec.tensor_add(z[:], op3[:], out2[:, t, :])
                    vec.tensor_add(z[:], z[:], b2f_bc[:])
                    o3 = lnp.tile([P, DM], F32, tag="o3")
                    layernorm(lnp, z, g3_bc, c3_bc, o3[:])
                    nc.sync.dma_start(dram["out3_s"][t], o3[:])


_NC_CACHE = None


def _get_nc():
    global _NC_CACHE
    if _NC_CACHE is None:
        _NC_CACHE = _build()
    return _NC_CACHE


def _make_in_maps(inputs):
    f = np.float32
    x = np.asarray(inputs["x"], f)
    enc = np.asarray(inputs["enc_output"], f)
    lam = np.asarray(inputs["look_ahead_mask"], f)[0, 0]   # [S, S]
    pm = np.ascontiguousarray(np.asarray(inputs["padding_mask"],
                                         f).reshape(1, S))

    shared = {}
    for l in ("1", "2"):
        for w in ("wq", "wk", "wv", "wo", "bq", "bk", "bv", "bo"):
            shared[f"mha{l}_{w}"] = np.ascontiguousarray(
                np.asarray(inputs[f"mha{l}_{w}"], f))
    for k in ("ffn_w1", "ffn_b1", "ffn_w2", "ffn_b2",
              "ln1_g", "ln1_b", "ln2_g", "ln2_b", "ln3_g", "ln3_b"):
        shared[k] = np.ascontiguousarray(np.asarray(inputs[k], f))

    in_maps = []
    for c in range(NC_COUNT):
        b, qi = c // 4, c % 4
        rows = np.concatenate([np.arange(128 * (qi + 4 * t),
                                         128 * (qi + 4 * t + 1))
                               for t in range(NT)])
        bmask = np.stack([lam[128 * (qi + 4 * t):128 * (qi + 4 * t + 1),
                              512 * t:512 * (t + 1)] for t in range(NT)])
        m = dict(shared)
        m["xb"] = np.ascontiguousarray(x[b])
        m["xq"] = np.ascontiguousarray(x[b][rows])
        m["encb"] = np.ascontiguousarray(enc[b])
        m["bmask"] = np.ascontiguousarray(bmask)
        m["pm"] = pm
        in_maps.append(m)
    return in_maps


def kernel(**inputs):
    nc = _get_nc()
    in_maps = _make_in_maps(inputs)
    res = run_bass_kernel_spmd(nc, in_maps, core_ids=list(range(NC_COUNT)))
    f = np.float32
    out3 = np.empty((B, S, DM), f)
    aw1 = np.empty((B, H, S, S), f)
    aw2 = np.empty((B, H, S, S), f)
    for c in range(NC_COUNT):
        b, qi = c // 4, c % 4
        r = res.results[c]
        for t in range(NT):
            g = qi + 4 * t
            sl = slice(128 * g, 128 * (g + 1))
            aw1[b, :, sl, :] = r["aw1_s"][:, t]
            aw2[b, :, sl, :] = r["aw2_s"][:, t]
            out3[b, sl, :] = r["out3_s"][t]
    return out3, aw1, aw2
